# revision 4
# baseline (speedup 1.0000x reference)
"""Trainium2 Bass kernel for 3-layer GCN (nn_MultiLayerGCN_48773648613817).

Strategy (8 NeuronCores, SPMD):
  - Nodes sharded across cores (12500/core, padded to 12544 = 98*128).
  - Per layer:  table = dis (.) (X @ W)  computed shard-local ([node,feat] rows),
    AllGather'd into a replicated DRAM table.
  - Edges partitioned by destination core, grouped into 128-node dest windows,
    padded to 128-edge chunks (layout shared across cores; per-core data).
  - Per chunk: indirect-DMA gather of 128 source rows (one 512B row per
    partition), one-hot selection matrix S built on DVE via is_equal against an
    iota row, TensorE matmul S^T @ msg accumulated into the window's PSUM.
  - Window epilogue: out = relu(dis * psum + b); layers 1-2 transpose back to
    X^T for the next layer's matmul, layer 3 DMAs rows to the output.

Self-loops are injected as ordinary edges (coefficient dis^2 = 1/deg matches
GCN's normalized self-loop exactly, since msg = dis[src]*h[src] and the window
epilogue multiplies by dis[dst]).
"""

import numpy as np

from concourse import bass, bacc, mybir, tile
from concourse.bass_utils import run_bass_kernel_spmd

N_NODES = 100000
N_LAYERS = 3
DIM = 128
N_CORES = 8
NSH = N_NODES // N_CORES          # 12500 real nodes per shard
P = 128
NWIN = 98                          # windows per shard
NSHP = NWIN * P                    # 12544 padded nodes per shard
N_TABLE = N_CORES * NSHP           # 100352 padded table rows

F32 = mybir.dt.float32
I32 = mybir.dt.int32


def _prepare(x, edge_indices, W, b):
    """Host-side index preprocessing. Returns (in_maps, layout) where layout
    gives the compile-time chunk counts per (layer, window), shared by all
    cores."""
    x = np.asarray(x, dtype=np.float32)
    ei = np.asarray(edge_indices).astype(np.int64)
    W = np.asarray(W, dtype=np.float32)
    b = np.asarray(b, dtype=np.float32)

    # per-core constant inputs
    iota_row = np.broadcast_to(
        np.arange(P, dtype=np.float32)[None, :], (P, P)
    ).copy()
    ident = np.eye(P, dtype=np.float32)
    bb = b.reshape(1, N_LAYERS * DIM).copy()

    xts = []
    for c in range(N_CORES):
        xs = x[c * NSH : (c + 1) * NSH]                      # [12500, 128]
        xp = np.zeros((NSHP, DIM), dtype=np.float32)
        xp[:NSH] = xs
        xts.append(np.ascontiguousarray(xp.T))               # [128, 12544]

    degs = np.ones((N_CORES, N_LAYERS, P, NWIN), dtype=np.float32)
    per_core_edges = [[None] * N_LAYERS for _ in range(N_CORES)]
    n_chunks = np.zeros((N_LAYERS, NWIN), dtype=np.int64)

    for l in range(N_LAYERS):
        row = ei[l, 0]
        col = ei[l, 1]
        deg = np.bincount(col, minlength=N_NODES).astype(np.float32) + 1.0
        src_pad = ((row // NSH) * NSHP + (row % NSH)).astype(np.int32)
        core_of = col // NSH
        lcol = (col % NSH).astype(np.int32)
        win = lcol // P
        dloc = (lcol % P).astype(np.float32)
        for c in range(N_CORES):
            m = core_of == c
            wc, dc, sc = win[m], dloc[m], src_pad[m]
            order = np.argsort(wc, kind="stable")
            wc, dc, sc = wc[order], dc[order], sc[order]
            cnt = np.bincount(wc, minlength=NWIN)
            per_core_edges[c][l] = (cnt, dc, sc)
            dlp = np.ones(NSHP, dtype=np.float32)
            dlp[:NSH] = deg[c * NSH : (c + 1) * NSH]
            degs[c, l] = dlp.reshape(NWIN, P).T
        cnts = np.stack([per_core_edges[c][l][0] for c in range(N_CORES)])
        n_chunks[l] = (cnts.max(axis=0) + P + P - 1) // P + 0  # +P self loops
        # ceil((max_cnt + 128)/128):
        n_chunks[l] = (cnts.max(axis=0) + P + (P - 1)) // P

    t_layer = n_chunks.sum(axis=1)             # chunks per layer
    tmax = int(t_layer.max())

    srcs_all = np.zeros((N_CORES, N_LAYERS, P, tmax), dtype=np.int32)
    dloc_all = np.full((N_CORES, N_LAYERS, P, tmax), -1.0, dtype=np.float32)
    for l in range(N_LAYERS):
        for c in range(N_CORES):
            cnt, dc, sc = per_core_edges[c][l]
            off = np.concatenate([[0], np.cumsum(cnt)[:-1]])
            tl = int(t_layer[l])
            s_arr = np.zeros((tl * P,), dtype=np.int32)
            d_arr = np.full((tl * P,), -1.0, dtype=np.float32)
            pos = 0
            for w in range(NWIN):
                k = int(n_chunks[l, w])
                nreal = int(cnt[w])
                # self loops first
                base = c * NSHP + w * P
                s_arr[pos : pos + P] = base + np.arange(P, dtype=np.int32)
                d_arr[pos : pos + P] = np.arange(P, dtype=np.float32)
                # real edges
                s_arr[pos + P : pos + P + nreal] = sc[off[w] : off[w] + nreal]
                d_arr[pos + P : pos + P + nreal] = dc[off[w] : off[w] + nreal]
                pos += k * P
            srcs_all[c, l, :, :tl] = s_arr.reshape(tl, P).T
            dloc_all[c, l, :, :tl] = d_arr.reshape(tl, P).T

    in_maps = []
    for c in range(N_CORES):
        in_maps.append(
            {
                "xt": xts[c],
                "wmat": W,
                "bb": bb,
                "iota": iota_row,
                "ident": ident,
                "degs": degs[c],
                "srcs": srcs_all[c],
                "dlocs": dloc_all[c],
            }
        )
    layout = (n_chunks, t_layer, tmax)
    return in_maps, layout


def _build(layout):
    n_chunks, t_layer, tmax = layout
    nc = bacc.Bacc(
        "TRN2", target_bir_lowering=False, debug=False, num_devices=N_CORES
    )
    xt_in = nc.dram_tensor("xt", [P, NSHP], F32, kind="ExternalInput").ap()
    w_in = nc.dram_tensor("wmat", [N_LAYERS, DIM, DIM], F32, kind="ExternalInput").ap()
    b_in = nc.dram_tensor("bb", [1, N_LAYERS * DIM], F32, kind="ExternalInput").ap()
    iota_in = nc.dram_tensor("iota", [P, P], F32, kind="ExternalInput").ap()
    id_in = nc.dram_tensor("ident", [P, P], F32, kind="ExternalInput").ap()
    deg_in = nc.dram_tensor("degs", [N_LAYERS, P, NWIN], F32, kind="ExternalInput").ap()
    srcs_in = nc.dram_tensor("srcs", [N_LAYERS, P, tmax], I32, kind="ExternalInput").ap()
    dloc_in = nc.dram_tensor("dlocs", [N_LAYERS, P, tmax], F32, kind="ExternalInput").ap()
    out_ap = nc.dram_tensor("out", [NSHP, DIM], F32, kind="ExternalOutput").ap()

    hloc = nc.dram_tensor("hloc", [NSHP, DIM], F32).ap()
    table = nc.dram_tensor("table", [N_TABLE, DIM], F32, addr_space="Shared").ap()

    with tile.TileContext(nc) as tc:
        with (
            tc.tile_pool(name="const", bufs=1) as constp,
            tc.tile_pool(name="xt", bufs=1) as xtp,
            tc.tile_pool(name="edges", bufs=2) as edgep,
            tc.tile_pool(name="msg", bufs=16) as msgp,
            tc.tile_pool(name="sel", bufs=2) as selp,
            tc.tile_pool(name="hsb", bufs=3) as hsbp,
            tc.tile_pool(name="tr", bufs=3) as trp,
            tc.tile_pool(name="ph", bufs=2, space="PSUM") as php,
            tc.tile_pool(name="pw", bufs=2, space="PSUM") as pwp,
            tc.tile_pool(name="pt", bufs=2, space="PSUM") as ptp,
            tc.tile_pool(name="pb", bufs=1, space="PSUM") as pbp,
        ):
            # constants
            iota_sb = constp.tile([P, P], F32)
            nc.sync.dma_start(out=iota_sb[:], in_=iota_in[:])
            ident_sb = constp.tile([P, P], F32)
            nc.sync.dma_start(out=ident_sb[:], in_=id_in[:])
            w_sb = constp.tile([P, N_LAYERS * DIM], F32)
            for l in range(N_LAYERS):
                nc.sync.dma_start(
                    out=w_sb[:, l * DIM : (l + 1) * DIM], in_=w_in[l]
                )
            brow_sb = constp.tile([1, N_LAYERS * DIM], F32)
            nc.sync.dma_start(out=brow_sb[:], in_=b_in[:])
            ones_row = constp.tile([1, P], F32)
            nc.vector.memset(ones_row[:], 1.0)

            xt_sb = xtp.tile([P, NSHP], F32)
            nc.sync.dma_start(out=xt_sb[:], in_=xt_in[:])

            for l in range(N_LAYERS):
                tl = int(t_layer[l])
                # --- normalization coefficients ---
                deg_sb = trp.tile([P, NWIN], F32, tag="deg")
                nc.sync.dma_start(out=deg_sb[:], in_=deg_in[l])
                dis_sb = trp.tile([P, NWIN], F32, tag="dis")
                nc.vector.reciprocal(dis_sb[:], deg_sb[:])
                nc.scalar.activation(
                    dis_sb[:], dis_sb[:], mybir.ActivationFunctionType.Sqrt
                )

                # --- b broadcast tile: ones_row^T (x) b_row ---
                pb = pbp.tile([P, P], F32, space="PSUM", tag="pb")
                nc.tensor.matmul(
                    out=pb[:],
                    lhsT=ones_row[:],
                    rhs=brow_sb[:, l * DIM : (l + 1) * DIM],
                    start=True,
                    stop=True,
                )
                bbc_sb = trp.tile([P, P], F32, tag="bbc")
                nc.vector.tensor_copy(out=bbc_sb[:], in_=pb[:])

                # --- H stage: table_local = dis * (X @ W) ---
                for w in range(NWIN):
                    ph = php.tile([P, P], F32, space="PSUM", tag="ph")
                    nc.tensor.matmul(
                        out=ph[:],
                        lhsT=xt_sb[:, w * P : (w + 1) * P],
                        rhs=w_sb[:, l * DIM : (l + 1) * DIM],
                        start=True,
                        stop=True,
                    )
                    ht = hsbp.tile([P, P], F32, tag="ht")
                    nc.vector.tensor_scalar(
                        out=ht[:],
                        in0=ph[:],
                        scalar1=dis_sb[:, w : w + 1],
                        scalar2=None,
                        op0=mybir.AluOpType.mult,
                    )
                    nc.sync.dma_start(out=hloc[w * P : (w + 1) * P, :], in_=ht[:])

                # --- replicate table ---
                nc.gpsimd.collective_compute(
                    "AllGather",
                    mybir.AluOpType.bypass,
                    replica_groups=[list(range(N_CORES))],
                    ins=[hloc[:]],
                    outs=[table[:]],
                )

                # --- edge metadata for this layer ---
                srcs_sb = edgep.tile([P, tl], I32, tag="srcs")
                nc.sync.dma_start(out=srcs_sb[:], in_=srcs_in[l, :, :tl])
                dloc_sb = edgep.tile([P, tl], F32, tag="dlocs")
                nc.sync.dma_start(out=dloc_sb[:], in_=dloc_in[l, :, :tl])

                # --- scatter stage ---
                t0 = 0
                for w in range(NWIN):
                    k = int(n_chunks[l, w])
                    # selection matrices for all chunks of this window
                    s_sb = selp.tile([P, k * P], F32, tag="sel")
                    nc.vector.tensor_tensor(
                        out=s_sb[:].rearrange("p (k j) -> p k j", k=k),
                        in0=dloc_sb[:, t0 : t0 + k].unsqueeze(2).to_broadcast([P, k, P]),
                        in1=iota_sb[:].unsqueeze(1).to_broadcast([P, k, P]),
                        op=mybir.AluOpType.is_equal,
                    )
                    pw = pwp.tile([P, P], F32, space="PSUM", tag="pw")
                    for j in range(k):
                        msg = msgp.tile([P, P], F32, tag="msg")
                        nc.gpsimd.indirect_dma_start(
                            out=msg[:],
                            out_offset=None,
                            in_=table[:],
                            in_offset=bass.IndirectOffsetOnAxis(
                                ap=srcs_sb[:, t0 + j : t0 + j + 1], axis=0
                            ),
                        )
                        nc.tensor.matmul(
                            out=pw[:],
                            lhsT=s_sb[:, j * P : (j + 1) * P],
                            rhs=msg[:],
                            start=(j == 0),
                            stop=(j == k - 1),
                        )
                    t0 += k

                    # --- window epilogue ---
                    t1 = trp.tile([P, P], F32, tag="t1")
                    nc.vector.tensor_scalar(
                        out=t1[:],
                        in0=pw[:],
                        scalar1=dis_sb[:, w : w + 1],
                        scalar2=None,
                        op0=mybir.AluOpType.mult,
                    )
                    nc.vector.tensor_tensor(
                        out=t1[:], in0=t1[:], in1=bbc_sb[:], op=mybir.AluOpType.add
                    )
                    if l < N_LAYERS - 1:
                        t2 = trp.tile([P, P], F32, tag="t2")
                        nc.vector.tensor_scalar(
                            out=t2[:],
                            in0=t1[:],
                            scalar1=0.0,
                            scalar2=None,
                            op0=mybir.AluOpType.max,
                        )
                        pt = ptp.tile([P, P], F32, space="PSUM", tag="pt")
                        nc.tensor.transpose(
                            out=pt[:], in_=t2[:], identity=ident_sb[:]
                        )
                        nc.vector.tensor_copy(
                            out=xt_sb[:, w * P : (w + 1) * P], in_=pt[:]
                        )
                    else:
                        t2 = trp.tile([P, P], F32, tag="t2")
                        nc.vector.tensor_scalar(
                            out=t2[:],
                            in0=t1[:],
                            scalar1=0.0,
                            scalar2=None,
                            op0=mybir.AluOpType.max,
                        )
                        nc.sync.dma_start(
                            out=out_ap[w * P : (w + 1) * P, :], in_=t2[:]
                        )

    nc.compile()
    return nc


def build_all(x, edge_indices, W, b):
    in_maps, layout = _prepare(x, edge_indices, W, b)
    nc = _build(layout)
    return nc, in_maps


def kernel(x, edge_indices, W, b):
    nc, in_maps = build_all(x, edge_indices, W, b)
    res = run_bass_kernel_spmd(nc, in_maps, list(range(N_CORES)))
    out = np.concatenate(
        [res.results[c]["out"][:NSH] for c in range(N_CORES)], axis=0
    )
    return out.astype(np.float32)


# revision 7
# speedup vs baseline: 1.3250x; 1.3250x over previous
"""Trainium2 Bass kernel for 3-layer GCN (nn_MultiLayerGCN_48773648613817).

Strategy (8 NeuronCores, SPMD):
  - Nodes sharded across cores (12500/core, padded to 12544 = 98*128).
  - Per layer:  table = dis (.) (X @ W)  computed shard-local ([node,feat] rows),
    AllGather'd into a replicated DRAM table.
  - Edges partitioned by destination core, grouped into 128-node dest windows,
    padded to 128-edge chunks (layout shared across cores; per-core data).
  - Per chunk: indirect-DMA gather of 128 source rows (one 512B row per
    partition), one-hot selection matrix S built on DVE via is_equal against an
    iota row, TensorE matmul S^T @ msg accumulated into the window's PSUM.
  - Window epilogue: out = relu(dis * psum + b); layers 1-2 transpose back to
    X^T for the next layer's matmul, layer 3 DMAs rows to the output.

Self-loops are injected as ordinary edges (coefficient dis^2 = 1/deg matches
GCN's normalized self-loop exactly, since msg = dis[src]*h[src] and the window
epilogue multiplies by dis[dst]).
"""

import numpy as np

from concourse import bass, bacc, mybir, tile
from concourse.bass_utils import run_bass_kernel_spmd

N_NODES = 100000
N_LAYERS = 3
DIM = 128
N_CORES = 8
NSH = N_NODES // N_CORES          # 12500 real nodes per shard
P = 128
NWIN = 98                          # windows per shard
NSHP = NWIN * P                    # 12544 padded nodes per shard
N_TABLE = N_CORES * NSHP           # 100352 padded table rows

F32 = mybir.dt.float32
BF = mybir.dt.bfloat16
I32 = mybir.dt.int32


def _prepare(x, edge_indices, W, b):
    """Host-side index preprocessing. Returns (in_maps, layout) where layout
    gives the compile-time chunk counts per (layer, window), shared by all
    cores."""
    x = np.asarray(x, dtype=np.float32)
    ei = np.asarray(edge_indices).astype(np.int64)
    W = np.asarray(W, dtype=np.float32)
    b = np.asarray(b, dtype=np.float32)

    import ml_dtypes
    BF16 = ml_dtypes.bfloat16
    # per-core constant inputs
    iota_row = np.broadcast_to(
        np.arange(P, dtype=np.float32)[None, :], (P, P)
    ).astype(BF16)
    ident = np.eye(P, dtype=np.float32)
    bb = b.reshape(1, N_LAYERS * DIM).copy()

    xts = []
    for c in range(N_CORES):
        xs = x[c * NSH : (c + 1) * NSH]                      # [12500, 128]
        xp = np.zeros((NSHP, DIM), dtype=np.float32)
        xp[:NSH] = xs
        xts.append(np.ascontiguousarray(xp.T))               # [128, 12544]

    degs = np.ones((N_CORES, N_LAYERS, P, NWIN), dtype=np.float32)
    per_core_edges = [[None] * N_LAYERS for _ in range(N_CORES)]
    n_chunks = np.zeros((N_LAYERS, NWIN), dtype=np.int64)

    for l in range(N_LAYERS):
        row = ei[l, 0]
        col = ei[l, 1]
        deg = np.bincount(col, minlength=N_NODES).astype(np.float32) + 1.0
        src_pad = ((row // NSH) * NSHP + (row % NSH)).astype(np.int32)
        core_of = col // NSH
        lcol = (col % NSH).astype(np.int32)
        win = lcol // P
        dloc = (lcol % P).astype(np.float32)
        for c in range(N_CORES):
            m = core_of == c
            wc, dc, sc = win[m], dloc[m], src_pad[m]
            order = np.argsort(wc, kind="stable")
            wc, dc, sc = wc[order], dc[order], sc[order]
            cnt = np.bincount(wc, minlength=NWIN)
            per_core_edges[c][l] = (cnt, dc, sc)
            dlp = np.ones(NSHP, dtype=np.float32)
            dlp[:NSH] = deg[c * NSH : (c + 1) * NSH]
            degs[c, l] = dlp.reshape(NWIN, P).T
        cnts = np.stack([per_core_edges[c][l][0] for c in range(N_CORES)])
        # self loops are handled in the window epilogue, not as edge chunks
        n_chunks[l] = np.maximum((cnts.max(axis=0) + (P - 1)) // P, 1)

    t_layer = n_chunks.sum(axis=1)             # chunks per layer
    tmax = int(t_layer.max())

    srcs_all = np.zeros((N_CORES, N_LAYERS, P, tmax), dtype=np.int32)
    dloc_all = np.full((N_CORES, N_LAYERS, P, tmax), -1.0, dtype=np.float32)
    # (dloc converted to bf16 at the end)
    for l in range(N_LAYERS):
        for c in range(N_CORES):
            cnt, dc, sc = per_core_edges[c][l]
            off = np.concatenate([[0], np.cumsum(cnt)[:-1]])
            tl = int(t_layer[l])
            s_arr = np.zeros((tl * P,), dtype=np.int32)
            d_arr = np.full((tl * P,), -1.0, dtype=np.float32)
            pos = 0
            for w in range(NWIN):
                k = int(n_chunks[l, w])
                nreal = int(cnt[w])
                s_arr[pos : pos + nreal] = sc[off[w] : off[w] + nreal]
                d_arr[pos : pos + nreal] = dc[off[w] : off[w] + nreal]
                pos += k * P
            srcs_all[c, l, :, :tl] = s_arr.reshape(tl, P).T
            dloc_all[c, l, :, :tl] = d_arr.reshape(tl, P).T

    in_maps = []
    for c in range(N_CORES):
        in_maps.append(
            {
                "xt": xts[c],
                "wmat": W,
                "bb": bb,
                "iota": iota_row,
                "ident": ident,
                "degs": degs[c],
                "srcs": srcs_all[c],
                "dlocs": dloc_all[c].astype(BF16),
            }
        )
    layout = (n_chunks, t_layer, tmax)
    return in_maps, layout


def _build(layout):
    n_chunks, t_layer, tmax = layout
    nc = bacc.Bacc(
        "TRN2", target_bir_lowering=False, debug=False, num_devices=N_CORES
    )
    xt_in = nc.dram_tensor("xt", [P, NSHP], F32, kind="ExternalInput").ap()
    w_in = nc.dram_tensor("wmat", [N_LAYERS, DIM, DIM], F32, kind="ExternalInput").ap()
    b_in = nc.dram_tensor("bb", [1, N_LAYERS * DIM], F32, kind="ExternalInput").ap()
    iota_in = nc.dram_tensor("iota", [P, P], BF, kind="ExternalInput").ap()
    id_in = nc.dram_tensor("ident", [P, P], F32, kind="ExternalInput").ap()
    deg_in = nc.dram_tensor("degs", [N_LAYERS, P, NWIN], F32, kind="ExternalInput").ap()
    srcs_in = nc.dram_tensor("srcs", [N_LAYERS, P, tmax], I32, kind="ExternalInput").ap()
    dloc_in = nc.dram_tensor("dlocs", [N_LAYERS, P, tmax], BF, kind="ExternalInput").ap()
    out_ap = nc.dram_tensor("out", [NSHP, DIM], F32, kind="ExternalOutput").ap()

    hloc = nc.dram_tensor("hloc", [NSHP, DIM], BF).ap()
    table = nc.dram_tensor("table", [N_TABLE, DIM], BF, addr_space="Shared").ap()

    with tile.TileContext(nc) as tc:
        with (
            tc.tile_pool(name="const", bufs=1) as constp,
            tc.tile_pool(name="xt", bufs=1) as xtp,
            tc.tile_pool(name="edges", bufs=2) as edgep,
            tc.tile_pool(name="msg", bufs=16) as msgp,
            tc.tile_pool(name="sel", bufs=2) as selp,
            tc.tile_pool(name="hsb", bufs=1) as hsbp,
            tc.tile_pool(name="tr", bufs=3) as trp,
            tc.tile_pool(name="ph", bufs=2, space="PSUM") as php,
            tc.tile_pool(name="pw", bufs=2, space="PSUM") as pwp,
            tc.tile_pool(name="pt", bufs=2, space="PSUM") as ptp,
            tc.tile_pool(name="pb", bufs=1, space="PSUM") as pbp,
        ):
            # constants
            iota_sb = constp.tile([P, P], BF)
            nc.sync.dma_start(out=iota_sb[:], in_=iota_in[:])
            ident_sb = constp.tile([P, P], F32)
            nc.sync.dma_start(out=ident_sb[:], in_=id_in[:])
            w_sb = constp.tile([P, N_LAYERS * DIM], F32)
            for l in range(N_LAYERS):
                nc.sync.dma_start(
                    out=w_sb[:, l * DIM : (l + 1) * DIM], in_=w_in[l]
                )
            brow_sb = constp.tile([1, N_LAYERS * DIM], F32)
            nc.sync.dma_start(out=brow_sb[:], in_=b_in[:])
            ones_row = constp.tile([1, P], F32)
            nc.vector.memset(ones_row[:], 1.0)

            xt_sb = xtp.tile([P, NSHP], F32)
            nc.sync.dma_start(out=xt_sb[:], in_=xt_in[:])

            for l in range(N_LAYERS):
                tl = int(t_layer[l])
                # --- normalization coefficients ---
                deg_sb = trp.tile([P, NWIN], F32, tag="deg")
                nc.sync.dma_start(out=deg_sb[:], in_=deg_in[l])
                dis_sb = trp.tile([P, NWIN], F32, tag="dis")
                nc.vector.reciprocal(dis_sb[:], deg_sb[:])
                nc.scalar.activation(
                    dis_sb[:], dis_sb[:], mybir.ActivationFunctionType.Sqrt
                )

                # --- b broadcast tile: ones_row^T (x) b_row ---
                pb = pbp.tile([P, P], F32, space="PSUM", tag="pb")
                nc.tensor.matmul(
                    out=pb[:],
                    lhsT=ones_row[:],
                    rhs=brow_sb[:, l * DIM : (l + 1) * DIM],
                    start=True,
                    stop=True,
                )
                bbc_sb = trp.tile([P, P], F32, tag="bbc")
                nc.vector.tensor_copy(out=bbc_sb[:], in_=pb[:])

                # --- H stage: table_local = dis * (X @ W) ---
                hsb = hsbp.tile([P, NWIN * P], BF, tag="hsb")
                for w in range(NWIN):
                    ph = php.tile([P, P], F32, space="PSUM", tag="ph")
                    nc.tensor.matmul(
                        out=ph[:],
                        lhsT=xt_sb[:, w * P : (w + 1) * P],
                        rhs=w_sb[:, l * DIM : (l + 1) * DIM],
                        start=True,
                        stop=True,
                    )
                    nc.vector.tensor_scalar(
                        out=hsb[:, w * P : (w + 1) * P],
                        in0=ph[:],
                        scalar1=dis_sb[:, w : w + 1],
                        scalar2=None,
                        op0=mybir.AluOpType.mult,
                    )
                nc.sync.dma_start(
                    out=hloc[:].rearrange("(w p) f -> p w f", p=P),
                    in_=hsb[:].rearrange("p (w f) -> p w f", f=DIM),
                )

                # --- replicate table ---
                nc.gpsimd.collective_compute(
                    "AllGather",
                    mybir.AluOpType.bypass,
                    replica_groups=[list(range(N_CORES))],
                    ins=[hloc[:]],
                    outs=[table[:]],
                )

                # --- edge metadata for this layer ---
                srcs_sb = edgep.tile([P, tl], I32, tag="srcs")
                nc.sync.dma_start(out=srcs_sb[:], in_=srcs_in[l, :, :tl])
                dloc_sb = edgep.tile([P, tl], BF, tag="dlocs")
                nc.sync.dma_start(out=dloc_sb[:], in_=dloc_in[l, :, :tl])

                # --- scatter stage ---
                t0 = 0
                for w in range(NWIN):
                    k = int(n_chunks[l, w])
                    # selection matrices for all chunks of this window
                    s_sb = selp.tile([P, k * P], BF, tag="sel")
                    nc.vector.tensor_tensor(
                        out=s_sb[:].rearrange("p (k j) -> p k j", k=k),
                        in0=dloc_sb[:, t0 : t0 + k].unsqueeze(2).to_broadcast([P, k, P]),
                        in1=iota_sb[:].unsqueeze(1).to_broadcast([P, k, P]),
                        op=mybir.AluOpType.is_equal,
                    )
                    pw = pwp.tile([P, P], F32, space="PSUM", tag="pw")
                    for j in range(k):
                        msg = msgp.tile([P, P], BF, tag="msg")
                        nc.gpsimd.indirect_dma_start(
                            out=msg[:],
                            out_offset=None,
                            in_=table[:],
                            in_offset=bass.IndirectOffsetOnAxis(
                                ap=srcs_sb[:, t0 + j : t0 + j + 1], axis=0
                            ),
                        )
                        nc.tensor.matmul(
                            out=pw[:],
                            lhsT=s_sb[:, j * P : (j + 1) * P],
                            rhs=msg[:],
                            start=(j == 0),
                            stop=(j == k - 1),
                        )
                    t0 += k

                    # --- window epilogue: out = relu(dis*(pw + h_self) + b) ---
                    t0g = trp.tile([P, P], F32, tag="t0g")
                    nc.vector.tensor_tensor(
                        out=t0g[:],
                        in0=pw[:],
                        in1=hsb[:, w * P : (w + 1) * P],
                        op=mybir.AluOpType.add,
                    )
                    t1 = trp.tile([P, P], F32, tag="t1")
                    nc.vector.tensor_scalar(
                        out=t1[:],
                        in0=t0g[:],
                        scalar1=dis_sb[:, w : w + 1],
                        scalar2=None,
                        op0=mybir.AluOpType.mult,
                    )
                    nc.vector.tensor_tensor(
                        out=t1[:], in0=t1[:], in1=bbc_sb[:], op=mybir.AluOpType.add
                    )
                    if l < N_LAYERS - 1:
                        t2 = trp.tile([P, P], F32, tag="t2")
                        nc.vector.tensor_scalar(
                            out=t2[:],
                            in0=t1[:],
                            scalar1=0.0,
                            scalar2=None,
                            op0=mybir.AluOpType.max,
                        )
                        pt = ptp.tile([P, P], F32, space="PSUM", tag="pt")
                        nc.tensor.transpose(
                            out=pt[:], in_=t2[:], identity=ident_sb[:]
                        )
                        nc.vector.tensor_copy(
                            out=xt_sb[:, w * P : (w + 1) * P], in_=pt[:]
                        )
                    else:
                        t2 = trp.tile([P, P], F32, tag="t2")
                        nc.vector.tensor_scalar(
                            out=t2[:],
                            in0=t1[:],
                            scalar1=0.0,
                            scalar2=None,
                            op0=mybir.AluOpType.max,
                        )
                        nc.sync.dma_start(
                            out=out_ap[w * P : (w + 1) * P, :], in_=t2[:]
                        )

    nc.compile()
    return nc


def build_all(x, edge_indices, W, b):
    in_maps, layout = _prepare(x, edge_indices, W, b)
    nc = _build(layout)
    return nc, in_maps


def kernel(x, edge_indices, W, b):
    nc, in_maps = build_all(x, edge_indices, W, b)
    res = run_bass_kernel_spmd(nc, in_maps, list(range(N_CORES)))
    out = np.concatenate(
        [res.results[c]["out"][:NSH] for c in range(N_CORES)], axis=0
    )
    return out.astype(np.float32)


# revision 8
# speedup vs baseline: 1.4076x; 1.0623x over previous
"""Trainium2 Bass kernel for 3-layer GCN (nn_MultiLayerGCN_48773648613817).

Strategy (8 NeuronCores, SPMD):
  - Nodes sharded across cores (12500/core, padded to 12544 = 98*128).
  - Per layer:  table = dis (.) (X @ W)  computed shard-local ([node,feat] rows),
    AllGather'd into a replicated DRAM table.
  - Edges partitioned by destination core, grouped into 128-node dest windows,
    padded to 128-edge chunks (layout shared across cores; per-core data).
  - Per chunk: indirect-DMA gather of 128 source rows (one 512B row per
    partition), one-hot selection matrix S built on DVE via is_equal against an
    iota row, TensorE matmul S^T @ msg accumulated into the window's PSUM.
  - Window epilogue: out = relu(dis * psum + b); layers 1-2 transpose back to
    X^T for the next layer's matmul, layer 3 DMAs rows to the output.

Self-loops are injected as ordinary edges (coefficient dis^2 = 1/deg matches
GCN's normalized self-loop exactly, since msg = dis[src]*h[src] and the window
epilogue multiplies by dis[dst]).
"""

import numpy as np

from concourse import bass, bacc, mybir, tile
from concourse.bass_utils import run_bass_kernel_spmd

N_NODES = 100000
N_LAYERS = 3
DIM = 128
N_CORES = 8
NSH = N_NODES // N_CORES          # 12500 real nodes per shard
P = 128
NWIN = 98                          # windows per shard
NSHP = NWIN * P                    # 12544 padded nodes per shard
N_TABLE = N_CORES * NSHP           # 100352 padded table rows

F32 = mybir.dt.float32
BF = mybir.dt.bfloat16
I32 = mybir.dt.int32


def _prepare(x, edge_indices, W, b):
    """Host-side index preprocessing. Returns (in_maps, layout) where layout
    gives the compile-time chunk counts per (layer, window), shared by all
    cores."""
    x = np.asarray(x, dtype=np.float32)
    ei = np.asarray(edge_indices).astype(np.int64)
    W = np.asarray(W, dtype=np.float32)
    b = np.asarray(b, dtype=np.float32)

    import ml_dtypes
    BF16 = ml_dtypes.bfloat16
    # per-core constant inputs
    iota_row = np.broadcast_to(
        np.arange(P, dtype=np.float32)[None, :], (P, P)
    ).astype(BF16)
    ident = np.eye(P, dtype=np.float32)
    bb = b.reshape(1, N_LAYERS * DIM).copy()

    xts = []
    for c in range(N_CORES):
        xs = x[c * NSH : (c + 1) * NSH]                      # [12500, 128]
        xp = np.zeros((NSHP, DIM), dtype=np.float32)
        xp[:NSH] = xs
        xts.append(np.ascontiguousarray(xp.T))               # [128, 12544]

    degs = np.ones((N_CORES, N_LAYERS, P, NWIN), dtype=np.float32)
    per_core_edges = [[None] * N_LAYERS for _ in range(N_CORES)]
    n_chunks = np.zeros((N_LAYERS, NWIN), dtype=np.int64)

    for l in range(N_LAYERS):
        row = ei[l, 0]
        col = ei[l, 1]
        deg = np.bincount(col, minlength=N_NODES).astype(np.float32) + 1.0
        src_pad = ((row // NSH) * NSHP + (row % NSH)).astype(np.int32)
        core_of = col // NSH
        lcol = (col % NSH).astype(np.int32)
        win = lcol // P
        dloc = (lcol % P).astype(np.float32)
        for c in range(N_CORES):
            m = core_of == c
            wc, dc, sc = win[m], dloc[m], src_pad[m]
            order = np.argsort(wc, kind="stable")
            wc, dc, sc = wc[order], dc[order], sc[order]
            cnt = np.bincount(wc, minlength=NWIN)
            per_core_edges[c][l] = (cnt, dc, sc)
            dlp = np.ones(NSHP, dtype=np.float32)
            dlp[:NSH] = deg[c * NSH : (c + 1) * NSH]
            degs[c, l] = dlp.reshape(NWIN, P).T
        cnts = np.stack([per_core_edges[c][l][0] for c in range(N_CORES)])
        # self loops are handled in the window epilogue, not as edge chunks
        n_chunks[l] = np.maximum((cnts.max(axis=0) + (P - 1)) // P, 1)

    t_layer = n_chunks.sum(axis=1)             # chunks per layer
    tmax = int(t_layer.max())

    srcs_all = np.zeros((N_CORES, N_LAYERS, P, tmax), dtype=np.int32)
    dloc_all = np.full((N_CORES, N_LAYERS, P, tmax), -1.0, dtype=np.float32)
    # (dloc converted to bf16 at the end)
    for l in range(N_LAYERS):
        for c in range(N_CORES):
            cnt, dc, sc = per_core_edges[c][l]
            off = np.concatenate([[0], np.cumsum(cnt)[:-1]])
            tl = int(t_layer[l])
            s_arr = np.zeros((tl * P,), dtype=np.int32)
            d_arr = np.full((tl * P,), -1.0, dtype=np.float32)
            pos = 0
            for w in range(NWIN):
                k = int(n_chunks[l, w])
                nreal = int(cnt[w])
                s_arr[pos : pos + nreal] = sc[off[w] : off[w] + nreal]
                d_arr[pos : pos + nreal] = dc[off[w] : off[w] + nreal]
                pos += k * P
            srcs_all[c, l, :, :tl] = s_arr.reshape(tl, P).T
            dloc_all[c, l, :, :tl] = d_arr.reshape(tl, P).T

    in_maps = []
    for c in range(N_CORES):
        in_maps.append(
            {
                "xt": xts[c],
                "wmat": W,
                "bb": bb,
                "iota": iota_row,
                "ident": ident,
                "degs": degs[c],
                "srcs": srcs_all[c],
                "dlocs": dloc_all[c].astype(BF16),
            }
        )
    layout = (n_chunks, t_layer, tmax)
    return in_maps, layout


def _build(layout, skip_collective=False, msg_bufs=16):
    n_chunks, t_layer, tmax = layout
    nc = bacc.Bacc(
        "TRN2", target_bir_lowering=False, debug=False, num_devices=N_CORES
    )
    xt_in = nc.dram_tensor("xt", [P, NSHP], F32, kind="ExternalInput").ap()
    w_in = nc.dram_tensor("wmat", [N_LAYERS, DIM, DIM], F32, kind="ExternalInput").ap()
    b_in = nc.dram_tensor("bb", [1, N_LAYERS * DIM], F32, kind="ExternalInput").ap()
    iota_in = nc.dram_tensor("iota", [P, P], BF, kind="ExternalInput").ap()
    id_in = nc.dram_tensor("ident", [P, P], F32, kind="ExternalInput").ap()
    deg_in = nc.dram_tensor("degs", [N_LAYERS, P, NWIN], F32, kind="ExternalInput").ap()
    srcs_in = nc.dram_tensor("srcs", [N_LAYERS, P, tmax], I32, kind="ExternalInput").ap()
    dloc_in = nc.dram_tensor("dlocs", [N_LAYERS, P, tmax], BF, kind="ExternalInput").ap()
    out_ap = nc.dram_tensor("out", [NSHP, DIM], F32, kind="ExternalOutput").ap()

    hloc = nc.dram_tensor("hloc", [NSHP, DIM], BF).ap()
    table = nc.dram_tensor("table", [N_TABLE, DIM], BF, addr_space="Shared").ap()

    with tile.TileContext(nc) as tc:
        with (
            tc.tile_pool(name="const", bufs=1) as constp,
            tc.tile_pool(name="xt", bufs=1) as xtp,
            tc.tile_pool(name="edges", bufs=2) as edgep,
            tc.tile_pool(name="msg", bufs=msg_bufs) as msgp,
            tc.tile_pool(name="sel", bufs=2) as selp,
            tc.tile_pool(name="hsb", bufs=1) as hsbp,
            tc.tile_pool(name="tr", bufs=3) as trp,
            tc.tile_pool(name="ph", bufs=2, space="PSUM") as php,
            tc.tile_pool(name="pw", bufs=2, space="PSUM") as pwp,
            tc.tile_pool(name="pt", bufs=2, space="PSUM") as ptp,
            tc.tile_pool(name="pb", bufs=1, space="PSUM") as pbp,
        ):
            # constants
            iota_sb = constp.tile([P, P], BF)
            nc.sync.dma_start(out=iota_sb[:], in_=iota_in[:])
            ident_sb = constp.tile([P, P], F32)
            nc.sync.dma_start(out=ident_sb[:], in_=id_in[:])
            w_sb = constp.tile([P, N_LAYERS * DIM], F32)
            for l in range(N_LAYERS):
                nc.sync.dma_start(
                    out=w_sb[:, l * DIM : (l + 1) * DIM], in_=w_in[l]
                )
            brow_sb = constp.tile([1, N_LAYERS * DIM], F32)
            nc.sync.dma_start(out=brow_sb[:], in_=b_in[:])
            ones_row = constp.tile([1, P], F32)
            nc.vector.memset(ones_row[:], 1.0)

            xt_sb = xtp.tile([P, NSHP], F32)
            nc.sync.dma_start(out=xt_sb[:], in_=xt_in[:])

            for l in range(N_LAYERS):
                tl = int(t_layer[l])
                # --- normalization coefficients ---
                deg_sb = trp.tile([P, NWIN], F32, tag="deg")
                nc.sync.dma_start(out=deg_sb[:], in_=deg_in[l])
                dis_sb = trp.tile([P, NWIN], F32, tag="dis")
                nc.vector.reciprocal(dis_sb[:], deg_sb[:])
                nc.scalar.activation(
                    dis_sb[:], dis_sb[:], mybir.ActivationFunctionType.Sqrt
                )

                # --- b broadcast tile: ones_row^T (x) b_row ---
                pb = pbp.tile([P, P], F32, space="PSUM", tag="pb")
                nc.tensor.matmul(
                    out=pb[:],
                    lhsT=ones_row[:],
                    rhs=brow_sb[:, l * DIM : (l + 1) * DIM],
                    start=True,
                    stop=True,
                )
                bbc_sb = trp.tile([P, P], F32, tag="bbc")
                nc.vector.tensor_copy(out=bbc_sb[:], in_=pb[:])

                # --- H stage: table_local = dis * (X @ W) ---
                hsb = hsbp.tile([P, NWIN * P], BF, tag="hsb")
                for w in range(NWIN):
                    ph = php.tile([P, P], F32, space="PSUM", tag="ph")
                    nc.tensor.matmul(
                        out=ph[:],
                        lhsT=xt_sb[:, w * P : (w + 1) * P],
                        rhs=w_sb[:, l * DIM : (l + 1) * DIM],
                        start=True,
                        stop=True,
                    )
                    nc.vector.tensor_scalar(
                        out=hsb[:, w * P : (w + 1) * P],
                        in0=ph[:],
                        scalar1=dis_sb[:, w : w + 1],
                        scalar2=None,
                        op0=mybir.AluOpType.mult,
                    )
                nc.sync.dma_start(
                    out=hloc[:].rearrange("(w p) f -> p w f", p=P),
                    in_=hsb[:].rearrange("p (w f) -> p w f", f=DIM),
                )

                # --- replicate table ---
                if skip_collective:
                    nc.sync.dma_start(out=table[:NSHP, :], in_=hloc[:])
                else:
                    nc.gpsimd.collective_compute(
                        "AllGather",
                        mybir.AluOpType.bypass,
                        replica_groups=[list(range(N_CORES))],
                        ins=[hloc[:]],
                        outs=[table[:]],
                    )

                # --- edge metadata for this layer ---
                srcs_sb = edgep.tile([P, tl], I32, tag="srcs")
                nc.sync.dma_start(out=srcs_sb[:], in_=srcs_in[l, :, :tl])
                dloc_sb = edgep.tile([P, tl], BF, tag="dlocs")
                nc.sync.dma_start(out=dloc_sb[:], in_=dloc_in[l, :, :tl])

                # --- scatter stage ---
                t0 = 0
                for w in range(NWIN):
                    k = int(n_chunks[l, w])
                    # selection matrices for all chunks of this window
                    s_sb = selp.tile([P, k * P], BF, tag="sel")
                    nc.vector.tensor_tensor(
                        out=s_sb[:].rearrange("p (k j) -> p k j", k=k),
                        in0=dloc_sb[:, t0 : t0 + k].unsqueeze(2).to_broadcast([P, k, P]),
                        in1=iota_sb[:].unsqueeze(1).to_broadcast([P, k, P]),
                        op=mybir.AluOpType.is_equal,
                    )
                    pw = pwp.tile([P, P], F32, space="PSUM", tag="pw")
                    for j in range(k):
                        msg = msgp.tile([P, P], BF, tag="msg")
                        nc.gpsimd.indirect_dma_start(
                            out=msg[:],
                            out_offset=None,
                            in_=table[:],
                            in_offset=bass.IndirectOffsetOnAxis(
                                ap=srcs_sb[:, t0 + j : t0 + j + 1], axis=0
                            ),
                        )
                        nc.tensor.matmul(
                            out=pw[:],
                            lhsT=s_sb[:, j * P : (j + 1) * P],
                            rhs=msg[:],
                            start=(j == 0),
                            stop=(j == k - 1),
                        )
                    t0 += k

                    # --- window epilogue: out = relu(dis*(pw + h_self) + b) ---
                    t0g = trp.tile([P, P], F32, tag="t0g")
                    nc.vector.tensor_tensor(
                        out=t0g[:],
                        in0=pw[:],
                        in1=hsb[:, w * P : (w + 1) * P],
                        op=mybir.AluOpType.add,
                    )
                    t1 = trp.tile([P, P], F32, tag="t1")
                    nc.vector.tensor_scalar(
                        out=t1[:],
                        in0=t0g[:],
                        scalar1=dis_sb[:, w : w + 1],
                        scalar2=None,
                        op0=mybir.AluOpType.mult,
                    )
                    nc.vector.tensor_tensor(
                        out=t1[:], in0=t1[:], in1=bbc_sb[:], op=mybir.AluOpType.add
                    )
                    if l < N_LAYERS - 1:
                        t2 = trp.tile([P, P], F32, tag="t2")
                        nc.vector.tensor_scalar(
                            out=t2[:],
                            in0=t1[:],
                            scalar1=0.0,
                            scalar2=None,
                            op0=mybir.AluOpType.max,
                        )
                        pt = ptp.tile([P, P], F32, space="PSUM", tag="pt")
                        nc.tensor.transpose(
                            out=pt[:], in_=t2[:], identity=ident_sb[:]
                        )
                        nc.vector.tensor_copy(
                            out=xt_sb[:, w * P : (w + 1) * P], in_=pt[:]
                        )
                    else:
                        t2 = trp.tile([P, P], F32, tag="t2")
                        nc.vector.tensor_scalar(
                            out=t2[:],
                            in0=t1[:],
                            scalar1=0.0,
                            scalar2=None,
                            op0=mybir.AluOpType.max,
                        )
                        nc.sync.dma_start(
                            out=out_ap[w * P : (w + 1) * P, :], in_=t2[:]
                        )

    nc.compile()
    return nc


def build_all(x, edge_indices, W, b):
    in_maps, layout = _prepare(x, edge_indices, W, b)
    nc = _build(layout)
    return nc, in_maps


def kernel(x, edge_indices, W, b):
    nc, in_maps = build_all(x, edge_indices, W, b)
    res = run_bass_kernel_spmd(nc, in_maps, list(range(N_CORES)))
    out = np.concatenate(
        [res.results[c]["out"][:NSH] for c in range(N_CORES)], axis=0
    )
    return out.astype(np.float32)
